# revision 26
# baseline (speedup 1.0000x reference)
"""Trainium2 Bass kernel for the conv-projected self-attention block.

Reference computation (B=8, C=256, N=64, K=256), per (b, n) slice X=[C,256]:
    q = wq X + bq; k = wk X + bk; v = wv X + bv
    s = k^T q / 16;  p = softmax(s, axis=l);  o = v p
    out = X + wp o + bp

Four GEMM stages, all fp8 (e4m3) DoubleRow (contraction 256 in one pass):

  1. G-trick:  s = X^T (wk^T wq) X + beta_l (+ query-consts that cancel in
     softmax).  G precomputed on host; beta_l injected into the score PSUM
     by one rank-2 masked DoubleRow matmul per key-half (bt/maskr tiles).
  2. U-trick:  wp (v p) = ((wp wv) X) p.  Ut = ((wp wv) X)^T computed
     transposed (X chunks stationary) so attention needs no transposes.
     bv folds into bp' = bp + wp bv, folded into the residual on host
     (xb = fp16(x + bp')).
  3. Deferred softmax normalization: ep = exp(s/16 - 3) in fp8 feeds the
     final matmul UNNORMALIZED; psw = sum_keys(ep) via an all-ones fp8 DR
     matmul; the final eviction applies out = psf * (1/sum) + xb, so no
     fp8 re-quantization of probabilities and no normalize pass on the
     PE-feeding path.

Sharding: data-parallel over B - one batch per NeuronCore (8 cores).

Engine split per pair of n-slices (balanced ~2.7us each):
  PE:    t = G X (2 DR MM N=512), Ut (4 DR MM N=256), scores (4 DR MM
         N=256 + 2 beta MM N=512), key-sum (1 DR MM N=512), final
         (4 DR MM N=256)  ~2.4us warm
  ACT:   t-evict [128,1024] + ut-evict-half [128,512] + exp [128,1024]
  DVE:   ut-evict-half [128,512] + recip [128,512] + 2x mult [128,512]
  GpSimd: residual add [128,1024] fp16 (SBUF-only)
PSUM: proj pool 2x2 banks + scores 2 + sum 1 + final 1 = 8 banks.
"""

import numpy as np
import ml_dtypes

import concourse.bass as bass
import concourse.bacc as bacc
import concourse.mybir as mybir
import concourse.tile as tile
from concourse.bass_utils import run_bass_kernel_spmd

F32 = mybir.dt.float32
F16 = mybir.dt.float16
FP8 = mybir.dt.float8e4
AF = mybir.ActivationFunctionType
ALU = mybir.AluOpType
DR = mybir.MatmulPerfMode.DoubleRow
E4 = ml_dtypes.float8_e4m3fn

B, C, N, K = 8, 256, 64, 256
GROUPS = [4, 4, 8, 8, 8, 8, 8, 8, 8]
assert sum(GROUPS) == N
SCALE = 1.0 / 16.0     # 1/sqrt(C) applied at exp
SHIFT = -3.0           # exp(s - 3): keeps ep in e4m3 range (max score ~7.7)

_CACHE = {}


def _build():
    nc = bacc.Bacc("TRN2", target_bir_lowering=False, debug=False,
                   num_devices=8)

    x8_d = nc.dram_tensor("x8", [C, N, K], FP8, kind="ExternalInput")
    xb_d = nc.dram_tensor("xb", [C, N, K], F16, kind="ExternalInput")
    gt_d = nc.dram_tensor("gt", [128, 2, C], FP8, kind="ExternalInput")
    wut_d = nc.dram_tensor("wut", [128, 2, C], FP8, kind="ExternalInput")
    bt_d = nc.dram_tensor("bt", [32, N // 2, 2, 2, 128], FP8,
                          kind="ExternalInput")
    mk_d = nc.dram_tensor("mk", [32, 2, 2, K], FP8, kind="ExternalInput")
    y_d = nc.dram_tensor("y", [C, N, K], F16, kind="ExternalOutput")

    with tile.TileContext(nc) as tc:
        with tc.tile_pool(name="const", bufs=1) as const, \
             tc.tile_pool(name="xg", bufs=2) as xgp, \
             tc.tile_pool(name="xh", bufs=2) as xhp, \
             tc.tile_pool(name="tg", bufs=3) as tgp, \
             tc.tile_pool(name="ut", bufs=3) as utp, \
             tc.tile_pool(name="sm", bufs=3) as smp, \
             tc.tile_pool(name="rc", bufs=2) as rcp, \
             tc.tile_pool(name="tf", bufs=2) as tfp, \
             tc.tile_pool(name="ot", bufs=3) as otp, \
             tc.tile_pool(name="ps_proj", bufs=2, space="PSUM") as ps_proj, \
             tc.tile_pool(name="ps_sc", bufs=1, space="PSUM") as ps_sc, \
             tc.tile_pool(name="ps_sum", bufs=1, space="PSUM") as ps_sum, \
             tc.tile_pool(name="ps_fin", bufs=1, space="PSUM") as ps_fin:

            # ---- constants needed by the first pair go first ----
            gt = const.tile([128, 2, C], FP8, name="gt")
            nc.sync.dma_start(out=gt[:, :, :], in_=gt_d[:, :, :])

            G0 = GROUPS[0]
            xg_first = xgp.tile([128, 2, G0, K], FP8, name="xg", tag="xg")
            for ci in range(2):
                nc.sync.dma_start(out=xg_first[:, ci, :, :],
                                  in_=x8_d[bass.ts(ci, 128), 0:G0, :])

            wut = const.tile([128, 2, C], FP8, name="wut")
            nc.sync.dma_start(out=wut[:, :, :], in_=wut_d[:, :, :])
            bt = const.tile([32, N // 2, 2, 2, 128], FP8, name="bt")
            nc.sync.dma_start(out=bt[:, :, :, :, :], in_=bt_d[:, :, :, :, :])
            maskr = const.tile([32, 2, 2, K], FP8, name="maskr")
            nc.sync.dma_start(out=maskr[:, :, :, :], in_=mk_d[:, :, :, :])
            ones8 = const.tile([128, 2, 128], FP8, name="ones8")
            nc.vector.memset(ones8, 1.0)
            nbias = const.tile([128, 1], F32, name="nbias")
            nc.vector.memset(nbias, SHIFT)
            wsc = const.tile([128, 2, 512], FP8, name="wsc")
            nc.vector.memset(wsc, 0.0)

            xh_first = xhp.tile([128, 2, G0, K], F16, name="xh", tag="xh")
            for ci in range(2):
                nc.sync.dma_start(out=xh_first[:, ci, :, :],
                                  in_=xb_d[bass.ts(ci, 128), 0:G0, :])

            # PE warmup: ~3.5us of dummy matmuls during the DMA head so the
            # HAM clock gate opens (K=8/8) before real work arrives.
            pswm = ps_sum.tile([128, 2, K], F32, name="pswm", tag="sum")
            for _ in range(8):
                nc.tensor.matmul(
                    pswm.rearrange("p a b -> p (a b)"), ones8, wsc,
                    start=True, stop=True, perf_mode=DR)

            def load_group(g, n0):
                G = GROUPS[g]
                xg = xgp.tile([128, 2, G, K], FP8, name="xg", tag="xg")
                xh = xhp.tile([128, 2, G, K], F16, name="xh", tag="xh")
                for ci in range(2):
                    nc.sync.dma_start(
                        out=xg[:, ci, :, :],
                        in_=x8_d[bass.ts(ci, 128), n0:n0 + G, :])
                for ci in range(2):
                    nc.sync.dma_start(
                        out=xh[:, ci, :, :],
                        in_=xb_d[bass.ts(ci, 128), n0:n0 + G, :])
                return xg, xh

            def proj_pair(sp0, xg):
                # --- t = G X for this pair: [128, 2(co), 2(sl), 256] fp8
                pst = ps_proj.tile([128, 2, 512], F32, name="pst", tag="proj")
                for co in range(2):
                    nc.tensor.matmul(
                        pst[:, co, :], gt[:, :, bass.ts(co, 128)],
                        xg[:, :, sp0:sp0 + 2, :].rearrange(
                            "p a b c -> p a (b c)"),
                        start=True, stop=True, perf_mode=DR)
                tg = tgp.tile([128, 2, 2, K], FP8, name="tg", tag="tg")
                nc.scalar.copy(tg.rearrange("p a b c -> p (a b c)"),
                               pst.rearrange("p a b -> p (a b)"))

                # --- Ut rows for this pair: [128, 4, 256] fp8 (key-major)
                psu = ps_proj.tile([128, 4, 256], F32, name="psu", tag="proj")
                for r in range(4):
                    nc.tensor.matmul(
                        psu[:, r, :],
                        xg[:, :, sp0 + r // 2, bass.ts(r % 2, 128)],
                        wut[:, :, :], start=True, stop=True, perf_mode=DR)
                ut = utp.tile([128, 4, C], FP8, name="ut", tag="ut")
                nc.scalar.copy(
                    ut[:, 0:2, :].rearrange("p a b -> p (a b)"),
                    psu[:, 0:2, :].rearrange("p a b -> p (a b)"))
                nc.vector.tensor_copy(
                    ut[:, 2:4, :].rearrange("p a b -> p (a b)"),
                    psu[:, 2:4, :].rearrange("p a b -> p (a b)"))
                return tg, ut

            def attention_pair(n0, sp0, xg, xh, tg, ut, add_eng):
                # --- scores: pss [128, 2(lt), 2(sp), 256]; bank = lt.  A DR
                # start=True clears has_written bank-wide, so per bank:
                # slice0 DR start=True, slice1 DR start=False (pending-zero
                # => overwrite), then ONE rank-2 beta matmul (N=512)
                # accumulates both slices via the 0/1 mask rows.
                pss = ps_sc.tile([128, 2, 2, K], F32, name="pss", tag="sc")
                for lt in range(2):
                    for sp in range(2):
                        nc.tensor.matmul(
                            pss[:, lt, sp, :],
                            xg[:, :, sp0 + sp, bass.ts(lt, 128)],
                            tg[:, :, sp, :],
                            start=(sp == 0), stop=False, perf_mode=DR,
                            skip_group_check=True)
                    nc.tensor.matmul(
                        pss[:, lt, :, :].rearrange("p a b -> p (a b)"),
                        bt[:, (n0 + sp0) // 2, lt, :, :],
                        maskr.rearrange("p a b c -> p a (b c)"),
                        start=False, stop=True, perf_mode=DR,
                        skip_group_check=True)
                ep = smp.tile([128, 2, 2, K], FP8, name="ep", tag="ep")
                nc.scalar.activation(
                    out=ep.rearrange("p a b c -> p (a b c)"),
                    in_=pss.rearrange("p a b c -> p (a b c)"),
                    func=AF.Exp, scale=SCALE, bias=nbias[:, 0:1])

                # --- key-sums (broadcast across partitions) + reciprocal
                psw = ps_sum.tile([128, 2, K], F32, name="psw", tag="sum")
                nc.tensor.matmul(
                    psw.rearrange("p a b -> p (a b)"), ones8,
                    ep.rearrange("p a b c -> p a (b c)"),
                    start=True, stop=True, perf_mode=DR)
                recip = rcp.tile([128, 2, K], F32, name="recip", tag="recip")
                nc.vector.reciprocal_approx_fast(out=recip, in_=psw)

                # --- final: psf = Ut^T ep (unnormalized), then
                # out = psf * (1/sum) on DVE, + residual on GpSimd/DVE.
                # tmpf/outf are ct-major so y DMA descriptors are 1KB.
                tmpf = tfp.tile([128, 2, 2, K], F16, name="tmpf", tag="tmpf")
                outf = otp.tile([128, 2, 2, K], F16, name="outf", tag="outf")
                for sp in range(2):
                    psf = ps_fin.tile([128, 2, K], F32, name="psf", tag="fin")
                    for ct in range(2):
                        nc.tensor.matmul(
                            psf[:, ct, :],
                            ut[:, 2 * sp:2 * sp + 2, bass.ts(ct, 128)],
                            ep[:, :, sp, :],
                            start=True, stop=True, perf_mode=DR)
                    nc.vector.tensor_tensor(
                        out=tmpf[:, :, sp, :], in0=psf,
                        in1=recip[:, sp:sp + 1, :].broadcast_to((128, 2, K)),
                        op=ALU.mult)
                    add_eng.tensor_tensor(
                        out=outf[:, :, sp, :],
                        in0=tmpf[:, :, sp, :],
                        in1=xh[:, :, sp0 + sp, :],
                        op=ALU.add)
                for ct in range(2):
                    nc.sync.dma_start(
                        out=y_d[bass.ts(ct, 128), n0 + sp0:n0 + sp0 + 2, :],
                        in_=outf[:, ct, :, :])

            # ---- software-pipelined pair loop: projections (stage A) run
            # one pair ahead of attention (stage B) so the ACT FIFO never
            # head-of-line-blocks ready t/ut evictions behind a stalled exp.
            ngroups = len(GROUPS)
            n_starts = [sum(GROUPS[:i]) for i in range(ngroups)]
            pairs = []          # (n0, sp0, g)
            for g in range(ngroups):
                for sp0 in range(0, GROUPS[g], 2):
                    pairs.append((n_starts[g], sp0, g))
            npairs = len(pairs)
            gdata = {0: (xg_first, xh_first)}
            proj = {}

            def emit_A(k):
                n0, sp0, g = pairs[k]
                if g not in gdata:
                    gdata[g] = load_group(g, n_starts[g])
                xg, _ = gdata[g]
                proj[k] = proj_pair(sp0, xg)

            emit_A(0)
            emit_A(1)
            for k in range(npairs):
                if k + 2 < npairs:
                    emit_A(k + 2)
                n0, sp0, g = pairs[k]
                xg, xh = gdata[g]
                tg, ut = proj.pop(k)
                add_eng = nc.vector if k >= npairs - 2 else nc.gpsimd
                attention_pair(n0, sp0, xg, xh, tg, ut, add_eng)

    nc.compile()
    return nc


def _get_nc():
    if "nc" not in _CACHE:
        _CACHE["nc"] = _build()
    return _CACHE["nc"]


def _host_prep(inputs):
    x = np.ascontiguousarray(np.asarray(inputs["x"]), dtype=np.float32)
    wq = np.asarray(inputs["wq"]).astype(np.float64)
    wk = np.asarray(inputs["wk"]).astype(np.float64)
    wv = np.asarray(inputs["wv"]).astype(np.float64)
    wp = np.asarray(inputs["wp"]).astype(np.float64)
    bq = np.asarray(inputs["bq"]).astype(np.float64)
    bv = np.asarray(inputs["bv"]).astype(np.float64)
    bp = np.asarray(inputs["bp"]).astype(np.float64)

    Gm = (wk.T @ wq).astype(np.float32)          # s = x^T G x (+beta)
    WU = (wp @ wv).astype(np.float32)            # out_pre = (WU x) p
    bpe = (bp + wp @ bv).astype(np.float32)      # v/final bias, into residual
    bvec = (wk.T @ bq).astype(np.float32)        # beta_l = bvec . x_l

    def dr_stationary(M):   # [c, co] layouts -> [128, 2, 256] DR tiles
        return np.ascontiguousarray(
            M.reshape(2, 128, C).transpose(1, 0, 2))

    gt8 = dr_stationary(np.ascontiguousarray(Gm.T).astype(E4))
    wut8 = dr_stationary(np.ascontiguousarray(WU.T).astype(E4))

    x8 = x.astype(E4)                            # [B, C, N, K]
    xb = (x + bpe[None, :, None, None]).astype(np.float16)
    # beta[b, n, l] then [N, 2, 128] per core
    beta = np.einsum('c,bcnk->bnk', bvec, x).astype(E4)
    bt8 = np.zeros((B, 32, N // 2, 2, 2, 128), dtype=E4)
    br = beta.reshape(B, N // 2, 2, 2, 128)   # [b, pair, sp, lt, l]
    bt8[:, 0, :, :, 0, :] = br[:, :, 0, :, :]
    bt8[:, 1, :, :, 0, :] = br[:, :, 1, :, :]
    mk8 = np.zeros((32, 2, 2, K), dtype=E4)
    mk8[0, :, 0, :] = 1.0
    mk8[1, :, 1, :] = 1.0
    return x8, xb, gt8, wut8, bt8, mk8


def run(inputs, trace=False):
    x8, xb, gt8, wut8, bt8, mk8 = _host_prep(inputs)
    nc = _get_nc()
    common = {"gt": gt8, "wut": wut8, "mk": mk8}
    in_maps = [dict(common, x8=x8[b], xb=xb[b], bt=bt8[b]) for b in range(B)]
    res = run_bass_kernel_spmd(nc, in_maps, core_ids=list(range(8)),
                               trace=trace)
    out = np.stack([res.results[b]["y"].astype(np.float32)
                    for b in range(B)], axis=0)
    return out, res


def kernel(**inputs):
    out, _ = run(inputs, trace=False)
    return out


# revision 28
# speedup vs baseline: 1.2015x; 1.2015x over previous
"""Trainium2 Bass kernel for the conv-projected self-attention block.

Reference computation (B=8, C=256, N=64, K=256), per (b, n) slice X=[C,256]:
    q = wq X + bq; k = wk X + bk; v = wv X + bv
    s = k^T q / 16;  p = softmax(s, axis=l);  o = v p
    out = X + wp o + bp

Four GEMM stages, all fp8 (e4m3) DoubleRow (contraction 256 in one pass):

  1. G-trick:  s = X^T (wk^T wq) X + beta_l (+ query-consts that cancel in
     softmax).  G precomputed on host; beta_l injected into the score PSUM
     by one rank-2 masked DoubleRow matmul per key-half (bt/maskr tiles).
  2. U-trick:  wp (v p) = ((wp wv) X) p.  Ut = ((wp wv) X)^T computed
     transposed (X chunks stationary) so attention needs no transposes.
     bv folds into bp' = bp + wp bv, folded into the residual on host
     (xb = fp16(x + bp')).
  3. Deferred softmax normalization: ep = exp(s/16 - 3) in fp8 feeds the
     final matmul UNNORMALIZED; psw = sum_keys(ep) via an all-ones fp8 DR
     matmul; the final eviction applies out = psf * (1/sum) + xb, so no
     fp8 re-quantization of probabilities and no normalize pass on the
     PE-feeding path.

Sharding: data-parallel over B - one batch per NeuronCore (8 cores).

Engine split per pair of n-slices (balanced ~2.7us each):
  PE:    t = G X (2 DR MM N=512), Ut (4 DR MM N=256), scores (4 DR MM
         N=256 + 2 beta MM N=512), key-sum (1 DR MM N=512), final
         (4 DR MM N=256)  ~2.4us warm
  ACT:   t-evict [128,1024] + ut-evict-half [128,512] + exp [128,1024]
  DVE:   ut-evict-half [128,512] + recip [128,512] + 2x mult [128,512]
  GpSimd: residual add [128,1024] fp16 (SBUF-only)
PSUM: proj pool 2x2 banks + scores 2 + sum 1 + final 1 = 8 banks.
"""

import numpy as np
import ml_dtypes

import concourse.bass as bass
import concourse.bacc as bacc
import concourse.mybir as mybir
import concourse.tile as tile
from concourse.bass_utils import run_bass_kernel_spmd

F32 = mybir.dt.float32
F16 = mybir.dt.float16
FP8 = mybir.dt.float8e4
AF = mybir.ActivationFunctionType
ALU = mybir.AluOpType
DR = mybir.MatmulPerfMode.DoubleRow
E4 = ml_dtypes.float8_e4m3fn

B, C, N, K = 8, 256, 64, 256
GROUPS = [2, 6, 8, 8, 8, 8, 8, 8, 8]
assert sum(GROUPS) == N
SCALE = 1.0 / 16.0     # 1/sqrt(C) applied at exp
SHIFT = -3.0           # exp(s - 3): keeps ep in e4m3 range (max score ~7.7)

_CACHE = {}


def _build():
    nc = bacc.Bacc("TRN2", target_bir_lowering=False, debug=False,
                   num_devices=8)

    x8_d = nc.dram_tensor("x8", [C, N, K], FP8, kind="ExternalInput")
    xb_d = nc.dram_tensor("xb", [C, N, K], F16, kind="ExternalInput")
    gt_d = nc.dram_tensor("gt", [128, 2, C], FP8, kind="ExternalInput")
    wut_d = nc.dram_tensor("wut", [128, 2, C], FP8, kind="ExternalInput")
    bt_d = nc.dram_tensor("bt", [32, N // 2, 2, 2, 128], FP8,
                          kind="ExternalInput")
    mk_d = nc.dram_tensor("mk", [32, 2, 2, K], FP8, kind="ExternalInput")
    y_d = nc.dram_tensor("y", [C, N, K], F16, kind="ExternalOutput")

    with tile.TileContext(nc) as tc:
        with tc.tile_pool(name="const", bufs=1) as const, \
             tc.tile_pool(name="xg", bufs=2) as xgp, \
             tc.tile_pool(name="xh", bufs=2) as xhp, \
             tc.tile_pool(name="tg", bufs=3) as tgp, \
             tc.tile_pool(name="ut", bufs=3) as utp, \
             tc.tile_pool(name="sm", bufs=3) as smp, \
             tc.tile_pool(name="rc", bufs=2) as rcp, \
             tc.tile_pool(name="tf", bufs=2) as tfp, \
             tc.tile_pool(name="ot", bufs=3) as otp, \
             tc.tile_pool(name="ps_proj", bufs=2, space="PSUM") as ps_proj, \
             tc.tile_pool(name="ps_sc", bufs=1, space="PSUM") as ps_sc, \
             tc.tile_pool(name="ps_sum", bufs=1, space="PSUM") as ps_sum, \
             tc.tile_pool(name="ps_fin", bufs=1, space="PSUM") as ps_fin:

            # ---- constants needed by the first pair go first ----
            gt = const.tile([128, 2, C], FP8, name="gt")
            nc.sync.dma_start(out=gt[:, :, :], in_=gt_d[:, :, :])

            G0 = GROUPS[0]
            xg_first = xgp.tile([128, 2, G0, K], FP8, name="xg", tag="xg")
            for ci in range(2):
                nc.sync.dma_start(out=xg_first[:, ci, :, :],
                                  in_=x8_d[bass.ts(ci, 128), 0:G0, :])

            wut = const.tile([128, 2, C], FP8, name="wut")
            nc.sync.dma_start(out=wut[:, :, :], in_=wut_d[:, :, :])
            bt = const.tile([32, N // 2, 2, 2, 128], FP8, name="bt")
            nc.sync.dma_start(out=bt[:, :, :, :, :], in_=bt_d[:, :, :, :, :])
            maskr = const.tile([32, 2, 2, K], FP8, name="maskr")
            nc.sync.dma_start(out=maskr[:, :, :, :], in_=mk_d[:, :, :, :])
            ones8 = const.tile([128, 2, 128], FP8, name="ones8")
            nc.vector.memset(ones8, 1.0)
            nbias = const.tile([128, 1], F32, name="nbias")
            nc.vector.memset(nbias, SHIFT)
            wsc = const.tile([128, 2, 512], FP8, name="wsc")
            nc.vector.memset(wsc, 0.0)

            xh_first = xhp.tile([128, 2, G0, K], F16, name="xh", tag="xh")
            for ci in range(2):
                nc.sync.dma_start(out=xh_first[:, ci, :, :],
                                  in_=xb_d[bass.ts(ci, 128), 0:G0, :])

            # PE warmup: ~3.5us of dummy matmuls during the DMA head so the
            # HAM clock gate opens (K=8/8) before real work arrives.
            pswm = ps_sum.tile([128, 2, K], F32, name="pswm", tag="sum")
            for _ in range(8):
                nc.tensor.matmul(
                    pswm.rearrange("p a b -> p (a b)"), ones8, wsc,
                    start=True, stop=True, perf_mode=DR)

            def load_group(g, n0):
                G = GROUPS[g]
                xg = xgp.tile([128, 2, G, K], FP8, name="xg", tag="xg")
                xh = xhp.tile([128, 2, G, K], F16, name="xh", tag="xh")
                for ci in range(2):
                    nc.sync.dma_start(
                        out=xg[:, ci, :, :],
                        in_=x8_d[bass.ts(ci, 128), n0:n0 + G, :])
                for ci in range(2):
                    nc.sync.dma_start(
                        out=xh[:, ci, :, :],
                        in_=xb_d[bass.ts(ci, 128), n0:n0 + G, :])
                return xg, xh

            def proj_pair(sp0, xg):
                # --- t = G X for this pair: [128, 2(co), 2(sl), 256] fp8
                pst = ps_proj.tile([128, 2, 512], F32, name="pst", tag="proj")
                for co in range(2):
                    nc.tensor.matmul(
                        pst[:, co, :], gt[:, :, bass.ts(co, 128)],
                        xg[:, :, sp0:sp0 + 2, :].rearrange(
                            "p a b c -> p a (b c)"),
                        start=True, stop=True, perf_mode=DR)
                tg = tgp.tile([128, 2, 2, K], FP8, name="tg", tag="tg")
                nc.scalar.copy(tg.rearrange("p a b c -> p (a b c)"),
                               pst.rearrange("p a b -> p (a b)"))

                # --- Ut rows for this pair: [128, 4, 256] fp8 (key-major)
                psu = ps_proj.tile([128, 4, 256], F32, name="psu", tag="proj")
                for r in range(4):
                    nc.tensor.matmul(
                        psu[:, r, :],
                        xg[:, :, sp0 + r // 2, bass.ts(r % 2, 128)],
                        wut[:, :, :], start=True, stop=True, perf_mode=DR)
                ut = utp.tile([128, 4, C], FP8, name="ut", tag="ut")
                nc.scalar.copy(
                    ut[:, 0:2, :].rearrange("p a b -> p (a b)"),
                    psu[:, 0:2, :].rearrange("p a b -> p (a b)"))
                nc.vector.tensor_copy(
                    ut[:, 2:4, :].rearrange("p a b -> p (a b)"),
                    psu[:, 2:4, :].rearrange("p a b -> p (a b)"))
                return tg, ut

            def attention_pair(n0, sp0, xg, xh, tg, ut, add_eng):
                # --- scores: pss [128, 2(lt), 2(sp), 256]; bank = lt.  A DR
                # start=True clears has_written bank-wide, so per bank:
                # slice0 DR start=True, slice1 DR start=False (pending-zero
                # => overwrite), then ONE rank-2 beta matmul (N=512)
                # accumulates both slices via the 0/1 mask rows.
                pss = ps_sc.tile([128, 2, 2, K], F32, name="pss", tag="sc")
                for lt in range(2):
                    for sp in range(2):
                        nc.tensor.matmul(
                            pss[:, lt, sp, :],
                            xg[:, :, sp0 + sp, bass.ts(lt, 128)],
                            tg[:, :, sp, :],
                            start=(sp == 0), stop=False, perf_mode=DR,
                            skip_group_check=True)
                    nc.tensor.matmul(
                        pss[:, lt, :, :].rearrange("p a b -> p (a b)"),
                        bt[:, (n0 + sp0) // 2, lt, :, :],
                        maskr.rearrange("p a b c -> p a (b c)"),
                        start=False, stop=True, perf_mode=DR,
                        skip_group_check=True)
                ep = smp.tile([128, 2, 2, K], FP8, name="ep", tag="ep")
                nc.scalar.activation(
                    out=ep.rearrange("p a b c -> p (a b c)"),
                    in_=pss.rearrange("p a b c -> p (a b c)"),
                    func=AF.Exp, scale=SCALE, bias=nbias[:, 0:1])

                # --- key-sums (broadcast across partitions) + reciprocal
                psw = ps_sum.tile([128, 2, K], F32, name="psw", tag="sum")
                nc.tensor.matmul(
                    psw.rearrange("p a b -> p (a b)"), ones8,
                    ep.rearrange("p a b c -> p a (b c)"),
                    start=True, stop=True, perf_mode=DR)
                recip = rcp.tile([128, 2, K], F32, name="recip", tag="recip")
                nc.vector.reciprocal_approx_fast(out=recip, in_=psw)

                # --- final: psf = Ut^T ep (unnormalized), then
                # out = psf * (1/sum) on DVE, + residual on GpSimd/DVE.
                # tmpf/outf are ct-major so y DMA descriptors are 1KB.
                tmpf = tfp.tile([128, 2, 2, K], F16, name="tmpf", tag="tmpf")
                outf = otp.tile([128, 2, 2, K], F16, name="outf", tag="outf")
                for sp in range(2):
                    psf = ps_fin.tile([128, 2, K], F32, name="psf", tag="fin")
                    for ct in range(2):
                        nc.tensor.matmul(
                            psf[:, ct, :],
                            ut[:, 2 * sp:2 * sp + 2, bass.ts(ct, 128)],
                            ep[:, :, sp, :],
                            start=True, stop=True, perf_mode=DR)
                    nc.vector.tensor_tensor(
                        out=tmpf[:, :, sp, :], in0=psf,
                        in1=recip[:, sp:sp + 1, :].broadcast_to((128, 2, K)),
                        op=ALU.mult)
                    add_eng.tensor_tensor(
                        out=outf[:, :, sp, :],
                        in0=tmpf[:, :, sp, :],
                        in1=xh[:, :, sp0 + sp, :],
                        op=ALU.add)
                for ct in range(2):
                    nc.sync.dma_start(
                        out=y_d[bass.ts(ct, 128), n0 + sp0:n0 + sp0 + 2, :],
                        in_=outf[:, ct, :, :])

            # ---- software-pipelined pair loop: projections (stage A) run
            # one pair ahead of attention (stage B) so the ACT FIFO never
            # head-of-line-blocks ready t/ut evictions behind a stalled exp.
            ngroups = len(GROUPS)
            n_starts = [sum(GROUPS[:i]) for i in range(ngroups)]
            pairs = []          # (n0, sp0, g)
            for g in range(ngroups):
                for sp0 in range(0, GROUPS[g], 2):
                    pairs.append((n_starts[g], sp0, g))
            npairs = len(pairs)
            gdata = {0: (xg_first, xh_first)}
            proj = {}

            def emit_A(k):
                n0, sp0, g = pairs[k]
                if g not in gdata:
                    gdata[g] = load_group(g, n_starts[g])
                xg, _ = gdata[g]
                proj[k] = proj_pair(sp0, xg)

            emit_A(0)
            emit_A(1)
            # spacer matmuls: fill the PE idle window while the pair-0
            # chain primes, so the HAM clock gate does not re-throttle.
            psfd = ps_fin.tile([128, 2, K], F32, name="psfd", tag="fin")
            for _ in range(6):
                nc.tensor.matmul(
                    psfd.rearrange("p a b -> p (a b)"), ones8, wsc,
                    start=True, stop=True, perf_mode=DR)
            for k in range(npairs):
                if k + 2 < npairs:
                    emit_A(k + 2)
                n0, sp0, g = pairs[k]
                xg, xh = gdata[g]
                tg, ut = proj.pop(k)
                add_eng = nc.vector if k >= npairs - 2 else nc.gpsimd
                attention_pair(n0, sp0, xg, xh, tg, ut, add_eng)

    nc.compile()
    return nc


def _get_nc():
    if "nc" not in _CACHE:
        _CACHE["nc"] = _build()
    return _CACHE["nc"]


def _host_prep(inputs):
    x = np.ascontiguousarray(np.asarray(inputs["x"]), dtype=np.float32)
    wq = np.asarray(inputs["wq"]).astype(np.float64)
    wk = np.asarray(inputs["wk"]).astype(np.float64)
    wv = np.asarray(inputs["wv"]).astype(np.float64)
    wp = np.asarray(inputs["wp"]).astype(np.float64)
    bq = np.asarray(inputs["bq"]).astype(np.float64)
    bv = np.asarray(inputs["bv"]).astype(np.float64)
    bp = np.asarray(inputs["bp"]).astype(np.float64)

    Gm = (wk.T @ wq).astype(np.float32)          # s = x^T G x (+beta)
    WU = (wp @ wv).astype(np.float32)            # out_pre = (WU x) p
    bpe = (bp + wp @ bv).astype(np.float32)      # v/final bias, into residual
    bvec = (wk.T @ bq).astype(np.float32)        # beta_l = bvec . x_l

    def dr_stationary(M):   # [c, co] layouts -> [128, 2, 256] DR tiles
        return np.ascontiguousarray(
            M.reshape(2, 128, C).transpose(1, 0, 2))

    gt8 = dr_stationary(np.ascontiguousarray(Gm.T).astype(E4))
    wut8 = dr_stationary(np.ascontiguousarray(WU.T).astype(E4))

    x8 = x.astype(E4)                            # [B, C, N, K]
    xb = (x + bpe[None, :, None, None]).astype(np.float16)
    # beta[b, n, l] then [N, 2, 128] per core
    beta = np.einsum('c,bcnk->bnk', bvec, x).astype(E4)
    bt8 = np.zeros((B, 32, N // 2, 2, 2, 128), dtype=E4)
    br = beta.reshape(B, N // 2, 2, 2, 128)   # [b, pair, sp, lt, l]
    bt8[:, 0, :, :, 0, :] = br[:, :, 0, :, :]
    bt8[:, 1, :, :, 0, :] = br[:, :, 1, :, :]
    mk8 = np.zeros((32, 2, 2, K), dtype=E4)
    mk8[0, :, 0, :] = 1.0
    mk8[1, :, 1, :] = 1.0
    return x8, xb, gt8, wut8, bt8, mk8


def run(inputs, trace=False):
    x8, xb, gt8, wut8, bt8, mk8 = _host_prep(inputs)
    nc = _get_nc()
    common = {"gt": gt8, "wut": wut8, "mk": mk8}
    in_maps = [dict(common, x8=x8[b], xb=xb[b], bt=bt8[b]) for b in range(B)]
    res = run_bass_kernel_spmd(nc, in_maps, core_ids=list(range(8)),
                               trace=trace)
    out = np.stack([res.results[b]["y"].astype(np.float32)
                    for b in range(B)], axis=0)
    return out, res


def kernel(**inputs):
    out, _ = run(inputs, trace=False)
    return out


# revision 29
# speedup vs baseline: 1.2319x; 1.0253x over previous
"""Trainium2 Bass kernel for the conv-projected self-attention block.

Reference computation (B=8, C=256, N=64, K=256), per (b, n) slice X=[C,256]:
    q = wq X + bq; k = wk X + bk; v = wv X + bv
    s = k^T q / 16;  p = softmax(s, axis=l);  o = v p
    out = X + wp o + bp

Four GEMM stages, all fp8 (e4m3) DoubleRow (contraction 256 in one pass):

  1. G-trick:  s = X^T (wk^T wq) X + beta_l (+ query-consts that cancel in
     softmax).  G precomputed on host; beta_l injected into the score PSUM
     by one rank-2 masked DoubleRow matmul per key-half (bt/maskr tiles).
  2. U-trick:  wp (v p) = ((wp wv) X) p.  Ut = ((wp wv) X)^T computed
     transposed (X chunks stationary) so attention needs no transposes.
     bv folds into bp' = bp + wp bv, folded into the residual on host
     (xb = fp16(x + bp')).
  3. Deferred softmax normalization: ep = exp(s/16 - 3) in fp8 feeds the
     final matmul UNNORMALIZED; psw = sum_keys(ep) via an all-ones fp8 DR
     matmul; the final eviction applies out = psf * (1/sum) + xb, so no
     fp8 re-quantization of probabilities and no normalize pass on the
     PE-feeding path.

Sharding: data-parallel over B - one batch per NeuronCore (8 cores).

Engine split per pair of n-slices (balanced ~2.7us each):
  PE:    t = G X (2 DR MM N=512), Ut (4 DR MM N=256), scores (4 DR MM
         N=256 + 2 beta MM N=512), key-sum (1 DR MM N=512), final
         (4 DR MM N=256)  ~2.4us warm
  ACT:   t-evict [128,1024] + ut-evict-half [128,512] + exp [128,1024]
  DVE:   ut-evict-half [128,512] + recip [128,512] + 2x mult [128,512]
  GpSimd: residual add [128,1024] fp16 (SBUF-only)
PSUM: proj pool 2x2 banks + scores 2 + sum 1 + final 1 = 8 banks.
"""

import numpy as np
import ml_dtypes

import concourse.bass as bass
import concourse.bacc as bacc
import concourse.mybir as mybir
import concourse.tile as tile
from concourse.bass_utils import run_bass_kernel_spmd

F32 = mybir.dt.float32
F16 = mybir.dt.float16
FP8 = mybir.dt.float8e4
AF = mybir.ActivationFunctionType
ALU = mybir.AluOpType
DR = mybir.MatmulPerfMode.DoubleRow
E4 = ml_dtypes.float8_e4m3fn

B, C, N, K = 8, 256, 64, 256
GROUPS = [2, 6, 8, 8, 8, 8, 8, 8, 8]
assert sum(GROUPS) == N
SCALE = 1.0 / 16.0     # 1/sqrt(C) applied at exp
SHIFT = -3.0           # exp(s - 3): keeps ep in e4m3 range (max score ~7.7)

_CACHE = {}


def _build():
    nc = bacc.Bacc("TRN2", target_bir_lowering=False, debug=False,
                   num_devices=8)

    x8_d = nc.dram_tensor("x8", [C, N, K], FP8, kind="ExternalInput")
    xb_d = nc.dram_tensor("xb", [C, N, K], F16, kind="ExternalInput")
    gt_d = nc.dram_tensor("gt", [128, 2, C], FP8, kind="ExternalInput")
    wut_d = nc.dram_tensor("wut", [128, 2, C], FP8, kind="ExternalInput")
    bt_d = nc.dram_tensor("bt", [32, N // 2, 2, 2, 128], FP8,
                          kind="ExternalInput")
    mk_d = nc.dram_tensor("mk", [32, 2, 2, K], FP8, kind="ExternalInput")
    y_d = nc.dram_tensor("y", [C, N, K], F16, kind="ExternalOutput")

    with tile.TileContext(nc) as tc:
        with tc.tile_pool(name="const", bufs=1) as const, \
             tc.tile_pool(name="xg", bufs=2) as xgp, \
             tc.tile_pool(name="xh", bufs=2) as xhp, \
             tc.tile_pool(name="tg", bufs=3) as tgp, \
             tc.tile_pool(name="ut", bufs=3) as utp, \
             tc.tile_pool(name="sm", bufs=3) as smp, \
             tc.tile_pool(name="rc", bufs=2) as rcp, \
             tc.tile_pool(name="tf", bufs=2) as tfp, \
             tc.tile_pool(name="ot", bufs=3) as otp, \
             tc.tile_pool(name="ps_proj", bufs=2, space="PSUM") as ps_proj, \
             tc.tile_pool(name="ps_sc", bufs=1, space="PSUM") as ps_sc, \
             tc.tile_pool(name="ps_sum", bufs=1, space="PSUM") as ps_sum, \
             tc.tile_pool(name="ps_fin", bufs=1, space="PSUM") as ps_fin:

            # ---- constants needed by the first pair go first ----
            gt = const.tile([128, 2, C], FP8, name="gt")
            nc.sync.dma_start(out=gt[:, :, :], in_=gt_d[:, :, :])

            G0 = GROUPS[0]
            xg_first = xgp.tile([128, 2, G0, K], FP8, name="xg", tag="xg")
            for ci in range(2):
                nc.sync.dma_start(out=xg_first[:, ci, :, :],
                                  in_=x8_d[bass.ts(ci, 128), 0:G0, :])

            wut = const.tile([128, 2, C], FP8, name="wut")
            nc.sync.dma_start(out=wut[:, :, :], in_=wut_d[:, :, :])
            bt = const.tile([32, N // 2, 2, 2, 128], FP8, name="bt")
            nc.sync.dma_start(out=bt[:, :, :, :, :], in_=bt_d[:, :, :, :, :])
            maskr = const.tile([32, 2, 2, K], FP8, name="maskr")
            nc.sync.dma_start(out=maskr[:, :, :, :], in_=mk_d[:, :, :, :])
            ones8 = const.tile([128, 2, 128], FP8, name="ones8")
            nc.vector.memset(ones8, 1.0)
            nbias = const.tile([128, 1], F32, name="nbias")
            nc.vector.memset(nbias, SHIFT)
            wsc = const.tile([128, 2, 512], FP8, name="wsc")
            nc.vector.memset(wsc, 0.0)

            xh_first = xhp.tile([128, 2, G0, K], F16, name="xh", tag="xh")
            for ci in range(2):
                nc.sync.dma_start(out=xh_first[:, ci, :, :],
                                  in_=xb_d[bass.ts(ci, 128), 0:G0, :])

            # PE warmup: ~3.5us of dummy matmuls during the DMA head so the
            # HAM clock gate opens (K=8/8) before real work arrives.
            pswm = ps_sum.tile([128, 2, K], F32, name="pswm", tag="sum")
            for _ in range(8):
                nc.tensor.matmul(
                    pswm.rearrange("p a b -> p (a b)"), ones8, wsc,
                    start=True, stop=True, perf_mode=DR)

            def load_group(g, n0):
                G = GROUPS[g]
                xg = xgp.tile([128, 2, G, K], FP8, name="xg", tag="xg")
                xh = xhp.tile([128, 2, G, K], F16, name="xh", tag="xh")
                for ci in range(2):
                    nc.sync.dma_start(
                        out=xg[:, ci, :, :],
                        in_=x8_d[bass.ts(ci, 128), n0:n0 + G, :])
                for ci in range(2):
                    nc.sync.dma_start(
                        out=xh[:, ci, :, :],
                        in_=xb_d[bass.ts(ci, 128), n0:n0 + G, :])
                return xg, xh

            def proj_pair(sp0, xg):
                # --- t = G X for this pair: [128, 2(co), 2(sl), 256] fp8
                pst = ps_proj.tile([128, 2, 512], F32, name="pst", tag="proj")
                for co in range(2):
                    nc.tensor.matmul(
                        pst[:, co, :], gt[:, :, bass.ts(co, 128)],
                        xg[:, :, sp0:sp0 + 2, :].rearrange(
                            "p a b c -> p a (b c)"),
                        start=True, stop=True, perf_mode=DR)
                tg = tgp.tile([128, 2, 2, K], FP8, name="tg", tag="tg")
                nc.scalar.copy(tg.rearrange("p a b c -> p (a b c)"),
                               pst.rearrange("p a b -> p (a b)"))

                # --- Ut rows for this pair: [128, 4, 256] fp8 (key-major)
                psu = ps_proj.tile([128, 4, 256], F32, name="psu", tag="proj")
                for r in range(4):
                    nc.tensor.matmul(
                        psu[:, r, :],
                        xg[:, :, sp0 + r // 2, bass.ts(r % 2, 128)],
                        wut[:, :, :], start=True, stop=True, perf_mode=DR)
                ut = utp.tile([128, 4, C], FP8, name="ut", tag="ut")
                nc.scalar.copy(
                    ut[:, 0:2, :].rearrange("p a b -> p (a b)"),
                    psu[:, 0:2, :].rearrange("p a b -> p (a b)"))
                nc.vector.tensor_copy(
                    ut[:, 2:4, :].rearrange("p a b -> p (a b)"),
                    psu[:, 2:4, :].rearrange("p a b -> p (a b)"))
                return tg, ut

            def attention_pair(n0, sp0, xg, xh, tg, ut, add_eng):
                # --- scores: pss [128, 2(lt), 2(sp), 256]; bank = lt.  A DR
                # start=True clears has_written bank-wide, so per bank:
                # slice0 DR start=True, slice1 DR start=False (pending-zero
                # => overwrite), then ONE rank-2 beta matmul (N=512)
                # accumulates both slices via the 0/1 mask rows.
                pss = ps_sc.tile([128, 2, 2, K], F32, name="pss", tag="sc")
                for lt in range(2):
                    for sp in range(2):
                        nc.tensor.matmul(
                            pss[:, lt, sp, :],
                            xg[:, :, sp0 + sp, bass.ts(lt, 128)],
                            tg[:, :, sp, :],
                            start=(sp == 0), stop=False, perf_mode=DR,
                            skip_group_check=True)
                    nc.tensor.matmul(
                        pss[:, lt, :, :].rearrange("p a b -> p (a b)"),
                        bt[:, (n0 + sp0) // 2, lt, :, :],
                        maskr.rearrange("p a b c -> p a (b c)"),
                        start=False, stop=True, perf_mode=DR,
                        skip_group_check=True)
                ep = smp.tile([128, 2, 2, K], FP8, name="ep", tag="ep")
                nc.scalar.activation(
                    out=ep.rearrange("p a b c -> p (a b c)"),
                    in_=pss.rearrange("p a b c -> p (a b c)"),
                    func=AF.Exp, scale=SCALE, bias=nbias[:, 0:1])

                # --- key-sums (broadcast across partitions) + reciprocal
                psw = ps_sum.tile([128, 2, K], F32, name="psw", tag="sum")
                nc.tensor.matmul(
                    psw.rearrange("p a b -> p (a b)"), ones8,
                    ep.rearrange("p a b c -> p a (b c)"),
                    start=True, stop=True, perf_mode=DR)
                recip = rcp.tile([128, 2, K], F32, name="recip", tag="recip")
                nc.vector.reciprocal_approx_fast(out=recip, in_=psw)

                # --- final: psf = Ut^T ep (unnormalized), then
                # out = psf * (1/sum) on DVE, + residual on GpSimd/DVE.
                # tmpf/outf are ct-major so y DMA descriptors are 1KB.
                tmpf = tfp.tile([128, 2, 2, K], F16, name="tmpf", tag="tmpf")
                outf = otp.tile([128, 2, 2, K], F16, name="outf", tag="outf")
                for sp in range(2):
                    psf = ps_fin.tile([128, 2, K], F32, name="psf", tag="fin")
                    for ct in range(2):
                        nc.tensor.matmul(
                            psf[:, ct, :],
                            ut[:, 2 * sp:2 * sp + 2, bass.ts(ct, 128)],
                            ep[:, :, sp, :],
                            start=True, stop=True, perf_mode=DR)
                    nc.vector.tensor_tensor(
                        out=tmpf[:, :, sp, :], in0=psf,
                        in1=recip[:, sp:sp + 1, :].broadcast_to((128, 2, K)),
                        op=ALU.mult)
                    add_eng.tensor_tensor(
                        out=outf[:, :, sp, :],
                        in0=tmpf[:, :, sp, :],
                        in1=xh[:, :, sp0 + sp, :],
                        op=ALU.add)
                for ct in range(2):
                    nc.sync.dma_start(
                        out=y_d[bass.ts(ct, 128), n0 + sp0:n0 + sp0 + 2, :],
                        in_=outf[:, ct, :, :])

            # ---- software-pipelined pair loop: projections (stage A) run
            # one pair ahead of attention (stage B) so the ACT FIFO never
            # head-of-line-blocks ready t/ut evictions behind a stalled exp.
            ngroups = len(GROUPS)
            n_starts = [sum(GROUPS[:i]) for i in range(ngroups)]
            pairs = []          # (n0, sp0, g)
            for g in range(ngroups):
                for sp0 in range(0, GROUPS[g], 2):
                    pairs.append((n_starts[g], sp0, g))
            npairs = len(pairs)
            gdata = {0: (xg_first, xh_first)}
            proj = {}

            def emit_A(k):
                n0, sp0, g = pairs[k]
                if g not in gdata:
                    gdata[g] = load_group(g, n_starts[g])
                xg, _ = gdata[g]
                proj[k] = proj_pair(sp0, xg)

            emit_A(0)
            emit_A(1)
            # spacer matmuls: fill the PE idle window while the pair-0
            # chain primes, so the HAM clock gate does not re-throttle.
            psfd = ps_fin.tile([128, 2, K], F32, name="psfd", tag="fin")
            for _ in range(6):
                nc.tensor.matmul(
                    psfd.rearrange("p a b -> p (a b)"), ones8, wsc,
                    start=True, stop=True, perf_mode=DR)
            for k in range(npairs):
                n0, sp0, g = pairs[k]
                xg, xh = gdata[g]
                tg, ut = proj.pop(k)
                add_eng = nc.vector if k >= npairs - 2 else nc.gpsimd
                attention_pair(n0, sp0, xg, xh, tg, ut, add_eng)
                if k + 2 < npairs:
                    emit_A(k + 2)

    nc.compile()
    return nc


def _get_nc():
    if "nc" not in _CACHE:
        _CACHE["nc"] = _build()
    return _CACHE["nc"]


def _host_prep(inputs):
    x = np.ascontiguousarray(np.asarray(inputs["x"]), dtype=np.float32)
    wq = np.asarray(inputs["wq"]).astype(np.float64)
    wk = np.asarray(inputs["wk"]).astype(np.float64)
    wv = np.asarray(inputs["wv"]).astype(np.float64)
    wp = np.asarray(inputs["wp"]).astype(np.float64)
    bq = np.asarray(inputs["bq"]).astype(np.float64)
    bv = np.asarray(inputs["bv"]).astype(np.float64)
    bp = np.asarray(inputs["bp"]).astype(np.float64)

    Gm = (wk.T @ wq).astype(np.float32)          # s = x^T G x (+beta)
    WU = (wp @ wv).astype(np.float32)            # out_pre = (WU x) p
    bpe = (bp + wp @ bv).astype(np.float32)      # v/final bias, into residual
    bvec = (wk.T @ bq).astype(np.float32)        # beta_l = bvec . x_l

    def dr_stationary(M):   # [c, co] layouts -> [128, 2, 256] DR tiles
        return np.ascontiguousarray(
            M.reshape(2, 128, C).transpose(1, 0, 2))

    gt8 = dr_stationary(np.ascontiguousarray(Gm.T).astype(E4))
    wut8 = dr_stationary(np.ascontiguousarray(WU.T).astype(E4))

    x8 = x.astype(E4)                            # [B, C, N, K]
    xb = (x + bpe[None, :, None, None]).astype(np.float16)
    # beta[b, n, l] then [N, 2, 128] per core
    beta = np.einsum('c,bcnk->bnk', bvec, x).astype(E4)
    bt8 = np.zeros((B, 32, N // 2, 2, 2, 128), dtype=E4)
    br = beta.reshape(B, N // 2, 2, 2, 128)   # [b, pair, sp, lt, l]
    bt8[:, 0, :, :, 0, :] = br[:, :, 0, :, :]
    bt8[:, 1, :, :, 0, :] = br[:, :, 1, :, :]
    mk8 = np.zeros((32, 2, 2, K), dtype=E4)
    mk8[0, :, 0, :] = 1.0
    mk8[1, :, 1, :] = 1.0
    return x8, xb, gt8, wut8, bt8, mk8


def run(inputs, trace=False):
    x8, xb, gt8, wut8, bt8, mk8 = _host_prep(inputs)
    nc = _get_nc()
    common = {"gt": gt8, "wut": wut8, "mk": mk8}
    in_maps = [dict(common, x8=x8[b], xb=xb[b], bt=bt8[b]) for b in range(B)]
    res = run_bass_kernel_spmd(nc, in_maps, core_ids=list(range(8)),
                               trace=trace)
    out = np.stack([res.results[b]["y"].astype(np.float32)
                    for b in range(B)], axis=0)
    return out, res


def kernel(**inputs):
    out, _ = run(inputs, trace=False)
    return out
